# revision 2
# baseline (speedup 1.0000x reference)
"""Trainium2 Bass kernel v2: 3D Gaussian mixture rendered on a voxel grid.

grid[z,y,x] = sum_a prod_axis (voxel-averaged 1D gaussian integrals via erf).

v2 strategy (changes vs v1 baseline):
  - Output grid still y-sharded: core i renders y-rows [16i, 16i+16).
  - Atom culling to 512 (4 blocks of 128) as before, PLUS y-ranged blocks:
    atoms are assigned to blocks so that block b only needs y-rows
    RANGES[b] of the slab (gaussian y-support fits inside). Cuts the
    Khatri-Rao H build + matmul columns from 64 to 48 row-equivalents.
  - gy (y-axis integrals) precomputed on host (fp16, amp*(0.5/vs)^3
    folded in) and broadcast-expanded on device to a contiguous
    gyrep[a, r, x] via a stride-0-source SBUF->SBUF DMA on the idle
    sync DMA queue. With gyrep contiguous, the H build tensor_tensor
    hits the DVE 2x_1P packed mode (measured 1211ns vs 2283ns per
    16-row block).
  - x/z erf on device (2 activation ops per block, bias_ptr per atom,
    edges pre-scaled by 1/(sqrt(2)*sigma) on host).
  - PE HAM warm-up: dummy matmuls during the input DMA flight so real
    matmuls run at 2.4 GHz instead of the cold 1.2 GHz.
  - fp16 output grid (host upcasts): halves output DMA; PSUM->SBUF
    copies are pure casts (amp already folded into gy).
"""

import os

import numpy as np

import concourse.bacc as bacc
import concourse.bass as bass
import concourse.tile as tile
from concourse import mybir
from concourse.bass_utils import run_bass_kernel_spmd

N_PIX = 128
N_CORES = 8
SLAB = N_PIX // N_CORES  # 16
NBLK = 4
CAP = NBLK * 128
MAXDIST = 6.5  # drop atoms farther than this (sigma) from the slab
SUPPORT = 3.5  # y-support radius (sigma) used for block range assignment
RANGES = [(0, 8), (0, 16), (4, 16), (8, 16)]  # y-rows rendered per block
_ROWS = [r1 - r0 for r0, r1 in RANGES]
_OFFS = [sum(_ROWS[:b]) for b in range(NBLK)]  # row offset of block b in gy48
NROWS = sum(_ROWS)  # 48

LAST_RESULTS = None

# f32 input column layout: bias cols then scaled edges
_C_BX = 0                      # NBLK cols: -pos_x*inv_d per block
_C_BZ = _C_BX + NBLK           # NBLK cols: -pos_z*inv_d
_C_EDGE = _C_BZ + NBLK         # 130 cols: edges*inv_d (129 real + 1 pad)
_W_IN = _C_EDGE + N_PIX + 2

# bank k (psum, y-rows [4k,4k+4)) -> blocks writing it
_BANK_WRITERS = [
    [b for b in range(NBLK) if RANGES[b][0] <= 4 * k and 4 * k + 4 <= RANGES[b][1]]
    for k in range(4)
]


def _bcast_mid(ap: bass.AP, n: int) -> bass.AP:
    """[128, F] AP -> [128, n, F] with a step-0 middle dim."""
    return bass.AP(
        tensor=ap.tensor, offset=ap.offset, ap=[ap.ap[0], [0, n], *ap.ap[1:]]
    )


def _bcast_inner(ap: bass.AP, n: int) -> bass.AP:
    """[128, F] AP -> [128, F, n] with a step-0 inner dim."""
    return bass.AP(
        tensor=ap.tensor, offset=ap.offset, ap=[*ap.ap, [0, n]]
    )


def _build_nc():
    f32 = mybir.dt.float32
    f16 = mybir.dt.float16
    Erf = mybir.ActivationFunctionType.Erf
    mult = mybir.AluOpType.mult

    nc = bacc.Bacc(None, target_bir_lowering=False, name="gauss3d2")
    inp_d = nc.dram_tensor("inp", [128, _W_IN], f32, kind="ExternalInput")
    gyrep_d = nc.dram_tensor("gyrep", [128, NROWS * N_PIX], f16, kind="ExternalInput")
    gy48_d = nc.dram_tensor("gy48", [128, NROWS], f16, kind="ExternalInput")
    grid_d = nc.dram_tensor("grid", [128, SLAB * N_PIX], f16, kind="ExternalOutput")

    with tile.TileContext(nc) as tc:
        with (
            tc.tile_pool(name="const", bufs=1) as const,
            tc.tile_pool(name="work", bufs=3) as work,
            tc.tile_pool(name="o", bufs=2) as opool,
            tc.tile_pool(name="ps", bufs=1, space="PSUM") as psum,
        ):
            # trigger the Erf ACT table load first (input DMA flight overlap)
            warm = const.tile([128, 1], f32)
            nc.scalar.activation(
                warm[:], nc.const_aps.scalar_like(0.0, warm[:]), Erf
            )

            # input DMAs: f32 ctl+edges on sync, gy48 on scalar
            inp = const.tile([128, _W_IN], f32)
            nc.sync.dma_start(inp[:], inp_d[:])
            sedges = inp[:, _C_EDGE : _C_EDGE + 130]
            bxs = inp[:, _C_BX : _C_BX + NBLK]
            bzs = inp[:, _C_BZ : _C_BZ + NBLK]
            gy48 = const.tile([128, NROWS], f16, name="gy48")
            nc.scalar.dma_start(gy48[:], gy48_d[:])

            # gyrep[a, r, x] (contiguous fp16 -> DVE 2x H build). Block 1's
            # slice comes pre-broadcast from the host, split over both DMA
            # queues; blocks 2/3 are broadcast-built on ACT in its idle
            # window after the erfs (deterministic, no HBM-variance).
            gyrep = const.tile([128, NROWS, N_PIX], f16, name="gyrep")
            o1, r1b = _OFFS[1], _ROWS[1]
            h1 = r1b // 2
            nc.sync.dma_start(
                gyrep[:, o1 : o1 + h1, :],
                gyrep_d[:, o1 * N_PIX : (o1 + h1) * N_PIX],
            )
            nc.scalar.dma_start(
                gyrep[:, o1 + h1 : o1 + r1b, :],
                gyrep_d[:, (o1 + h1) * N_PIX : (o1 + r1b) * N_PIX],
            )

            # PE HAM warm-up fodder (no input deps): junk weights + rhs
            wgt = const.tile([128, 128], f16, name="wgt")
            wrhs = const.tile([128, 512], f16, name="wrhs")
            nc.vector.memset(wgt[:], 0.0)
            nc.vector.memset(wrhs[:], 0.0)
            pswarm = psum.tile([128, 512], f32, tag="pswarm", name="pswarm")
            for _ in range(12):
                nc.tensor.matmul(
                    pswarm[:], lhsT=wgt[:], rhs=wrhs[:],
                    start=True, stop=True, skip_group_check=True,
                )

            # 4 psum banks; banks 0,1 = y-rows 0-7 (written by B0,B1,B2),
            # banks 2,3 = y-rows 8-15 (written by B1,B2,B3)
            pss = [
                psum.tile([128, 512], f32, tag=f"ps{k}", name=f"ps{k}")
                for k in range(4)
            ]
            _PS_WRITERS = [
                [
                    b
                    for b in range(NBLK)
                    if RANGES[b][0] <= 4 * k and 4 * k + 4 <= RANGES[b][1]
                ]
                for k in range(4)
            ]
            shipped = set()

            def ship_bank(k):
                # copy engine alternates by bank; DMAs on both queues; the
                # final pair ships in halves so the last (receipt-latency-
                # bound) DMA is small and issued as early as possible
                eng = "a" if k in (0, 1, 2) else "v"
                ot = opool.tile([128, 512], f16, tag=f"ot{k}", name=f"ot{k}")
                q = nc.sync if eng == "a" else nc.scalar
                parts = ((0, 256), (256, 512)) if k >= 2 else ((0, 512),)
                for lo, hi in parts:
                    if eng == "a":
                        nc.scalar.copy(ot[:, lo:hi], pss[k][:, lo:hi])
                    else:
                        nc.vector.tensor_copy(ot[:, lo:hi], pss[k][:, lo:hi])
                    q.dma_start(
                        grid_d[:, 512 * k + lo : 512 * k + hi], ot[:, lo:hi]
                    )
                shipped.add(k)

            # all erfs first (ACT), so ACT's idle window after them can
            # build gyrep for blocks 2/3
            exzs = []
            for b in range(NBLK):
                exz = work.tile([128, 260], f32, tag=f"exz{b}", name=f"exz{b}")
                nc.scalar.activation(
                    exz[:, 0:130], sedges, Erf, bias=bxs[:, b : b + 1], scale=1.0
                )
                nc.scalar.activation(
                    exz[:, 130:260], sedges, Erf, bias=bzs[:, b : b + 1], scale=1.0
                )
                exzs.append(exz)
            for b in (2, 3):
                o, r = _OFFS[b], _ROWS[b]
                nc.scalar.copy(
                    gyrep[:, o : o + r, :],
                    gy48[:, o : o + r].broadcast_to([128, r, N_PIX]),
                )

            for b in range(NBLK):
                r0, r1 = RANGES[b]
                rows = r1 - r0
                exz = exzs[b]
                # diff: gx at [0:128], gz at [130:258] (junk at 128,129,258,259)
                gxz = work.tile([128, 260], f16, tag=f"gxz{b}", name=f"gxz{b}")
                nc.vector.tensor_sub(gxz[:, 0:259], exz[:, 1:260], exz[:, 0:259])
                gx = gxz[:, 0:N_PIX]
                gz = gxz[:, 130 : 130 + N_PIX]

                # H[a, r, x] = gx[a,x] * gy[a,r]
                h = work.tile([128, rows, N_PIX], f16, tag=f"h{b}", name=f"h{b}")
                o = _OFFS[b]
                if b == 0:
                    # 1x broadcast from the small gy48 (no bulk-DMA wait)
                    nc.vector.tensor_tensor(
                        h[:],
                        _bcast_mid(gx, rows),
                        gy48[:, o : o + rows].broadcast_to([128, rows, N_PIX]),
                        mult,
                    )
                else:
                    # DVE 2x packed mode via contiguous gyrep
                    nc.vector.tensor_tensor(
                        h[:], _bcast_mid(gx, rows), gyrep[:, o : o + rows, :], mult
                    )

                # matmuls: bank k covers y-rows [4k, 4k+4)
                for k in range(4):
                    if b not in _PS_WRITERS[k]:
                        continue
                    first = _PS_WRITERS[k][0] == b
                    last = _PS_WRITERS[k][-1] == b
                    lo = 4 * k - r0
                    nc.tensor.matmul(
                        pss[k][:],
                        lhsT=gz,
                        rhs=h[:, lo : lo + 4, :],
                        start=first,
                        stop=last,
                        skip_group_check=True,
                    )
                for k in range(4):
                    if k not in shipped and _PS_WRITERS[k][-1] == b:
                        ship_bank(k)

    nc.compile()
    return nc


def _shard_inputs(pos: np.ndarray, sigma: float, vs: float, n_pix: int, c_amp: float):
    """Per-core inputs: f32 [128, _W_IN] (bias + scaled edges) + f16 gy48."""
    from scipy.special import erf as serf

    inv_d = np.float64(1.0 / (np.sqrt(2.0) * sigma))
    edges = ((np.arange(n_pix + 1, dtype=np.float64) - n_pix // 2) - 0.5) * vs
    sedges = np.zeros((130,), np.float64)
    sedges[: n_pix + 1] = edges * inv_d
    sedges[n_pix + 1] = sedges[n_pix]  # pad col

    in_maps = []
    for i in range(N_CORES):
        lo, hi = edges[SLAB * i], edges[SLAB * i + SLAB]
        py = pos[:, 1].astype(np.float64)
        d = np.maximum(0.0, np.maximum(lo - py, py - hi))
        idx = np.argsort(d, kind="stable")[:CAP]
        idx = idx[d[idx] <= MAXDIST * sigma]

        centers = (np.arange(SLAB, dtype=np.float64) + SLAB * i - n_pix // 2) * vs
        p = py[idx]
        first = np.searchsorted(centers, p - SUPPORT * sigma, side="left")
        last = np.searchsorted(centers, p + SUPPORT * sigma, side="right")
        elig = [
            [
                b
                for b, (r0, r1) in enumerate(RANGES)
                if first[a] >= r0 and last[a] <= r1
            ]
            for a in range(len(idx))
        ]
        order = sorted(range(len(idx)), key=lambda a: len(elig[a]))
        cap = [128] * NBLK
        assign = [[] for _ in range(NBLK)]
        for a in order:
            cands = [b for b in elig[a] if cap[b] > 0]
            assert cands, f"core {i}: atom unassignable"
            b = min(cands, key=lambda b: RANGES[b][1] - RANGES[b][0])
            cap[b] -= 1
            assign[b].append(idx[a])

        buf = np.zeros((128, _W_IN), np.float32)
        buf[:, _C_EDGE : _C_EDGE + 130] = sedges[None, :].astype(np.float32)
        gy48 = np.zeros((128, NROWS), np.float16)
        yed = edges[SLAB * i : SLAB * i + SLAB + 1]
        for b in range(NBLK):
            r0, r1 = RANGES[b]
            sel = np.array(assign[b], dtype=np.int64)
            n = len(sel)
            px = np.zeros((128,), np.float64)
            pz = np.zeros((128,), np.float64)
            if n:
                px[:n] = pos[sel, 0]
                pz[:n] = pos[sel, 2]
                E = serf((yed[None, r0 : r1 + 1] - py[sel][:, None]) * inv_d)
                g = (E[:, 1:] - E[:, :-1]) * c_amp  # c_amp = amp*(0.5/vs)^3
                gy48[:n, _OFFS[b] : _OFFS[b] + (r1 - r0)] = g.astype(np.float16)
            buf[:, _C_BX + b] = (-px * inv_d).astype(np.float32)
            buf[:, _C_BZ + b] = (-pz * inv_d).astype(np.float32)
        gyrep = np.repeat(gy48[:, :, None], N_PIX, axis=2).reshape(128, NROWS * N_PIX)
        in_maps.append({"inp": buf, "gyrep": gyrep, "gy48": gy48})
    return in_maps


def kernel(
    atom_positions: np.ndarray,
    log_var: np.ndarray,
    log_weight: np.ndarray,
    n_pix,
    voxel_size,
) -> np.ndarray:
    global LAST_RESULTS
    pos = np.asarray(atom_positions, dtype=np.float32)
    lv = float(np.asarray(log_var, dtype=np.float32).reshape(-1)[0])
    lw = float(np.asarray(log_weight, dtype=np.float32).reshape(-1)[0])
    n_pix = int(n_pix)
    vs = float(voxel_size)
    assert n_pix == N_PIX, f"kernel compiled for n_pix={N_PIX}, got {n_pix}"

    sigma = float(np.exp(0.5 * lv))
    amp = float(np.exp(lw))
    c_amp = float(amp * (0.5 / vs) ** 3)  # folded into gy on host

    in_maps = _shard_inputs(pos, sigma, vs, n_pix, c_amp)
    nc = _build_nc()
    res = run_bass_kernel_spmd(
        nc,
        in_maps,
        core_ids=list(range(N_CORES)),
        trace=bool(int(os.environ.get("GAUSS3D_TRACE", "0"))),
    )
    LAST_RESULTS = res
    grids = [
        r["grid"].astype(np.float32).reshape(N_PIX, SLAB, N_PIX) for r in res.results
    ]
    return np.ascontiguousarray(np.concatenate(grids, axis=1), dtype=np.float32)


# revision 4
# speedup vs baseline: 1.0023x; 1.0023x over previous
"""Trainium2 Bass kernel v2: 3D Gaussian mixture rendered on a voxel grid.

grid[z,y,x] = sum_a prod_axis (voxel-averaged 1D gaussian integrals via erf).

v2 strategy (changes vs v1 baseline):
  - Output grid still y-sharded: core i renders y-rows [16i, 16i+16).
  - Atom culling to 512 (4 blocks of 128) as before, PLUS y-ranged blocks:
    atoms are assigned to blocks so that block b only needs y-rows
    RANGES[b] of the slab (gaussian y-support fits inside). Cuts the
    Khatri-Rao H build + matmul columns from 64 to 48 row-equivalents.
  - gy (y-axis integrals) precomputed on host (fp16, amp*(0.5/vs)^3
    folded in). For the H build, a contiguous gyrep[a, r, x] makes the
    tensor_tensor hit the DVE 2x_1P packed mode (1211 vs 2283ns per
    16-row block): block 1's gyrep slice is DMA'd pre-broadcast from
    the host (split over both HWDGE queues), blocks 2/3 are broadcast-
    built by ScalarE in its idle window after the erfs, and block 0
    (first, latency-critical) runs at 1x straight from the small gy48.
  - x/z erf on device (2 activation ops per block, bias_ptr per atom,
    edges pre-scaled by 1/(sqrt(2)*sigma) on host).
  - PE HAM warm-up: dummy matmuls during the input DMA flight so real
    matmuls run at 2.4 GHz instead of the cold 1.2 GHz.
  - fp16 output grid (host upcasts): halves output DMA; PSUM->SBUF
    copies are pure casts (amp already folded into gy).
"""

import os

import numpy as np

import concourse.bacc as bacc
import concourse.bass as bass
import concourse.tile as tile
from concourse import mybir
from concourse.bass_utils import run_bass_kernel_spmd

N_PIX = 128
N_CORES = 8
SLAB = N_PIX // N_CORES  # 16
NBLK = 4
CAP = NBLK * 128
MAXDIST = 6.5  # drop atoms farther than this (sigma) from the slab
SUPPORT = 3.5  # y-support radius (sigma) used for block range assignment
RANGES = [(0, 8), (0, 16), (4, 16), (8, 16)]  # y-rows rendered per block
_ROWS = [r1 - r0 for r0, r1 in RANGES]
_OFFS = [sum(_ROWS[:b]) for b in range(NBLK)]  # row offset of block b in gy48
NROWS = sum(_ROWS)  # 48

LAST_RESULTS = None

# f32 input column layout: bias cols then scaled edges
_C_BX = 0                      # NBLK cols: -pos_x*inv_d per block
_C_BZ = _C_BX + NBLK           # NBLK cols: -pos_z*inv_d
_C_EDGE = _C_BZ + NBLK         # 130 cols: edges*inv_d (129 real + 1 pad)
_W_IN = _C_EDGE + N_PIX + 2

# bank k (psum, y-rows [4k,4k+4)) -> blocks writing it
_BANK_WRITERS = [
    [b for b in range(NBLK) if RANGES[b][0] <= 4 * k and 4 * k + 4 <= RANGES[b][1]]
    for k in range(4)
]


def _bcast_mid(ap: bass.AP, n: int) -> bass.AP:
    """[128, F] AP -> [128, n, F] with a step-0 middle dim."""
    return bass.AP(
        tensor=ap.tensor, offset=ap.offset, ap=[ap.ap[0], [0, n], *ap.ap[1:]]
    )


def _bcast_inner(ap: bass.AP, n: int) -> bass.AP:
    """[128, F] AP -> [128, F, n] with a step-0 inner dim."""
    return bass.AP(
        tensor=ap.tensor, offset=ap.offset, ap=[*ap.ap, [0, n]]
    )


def _build_nc():
    f32 = mybir.dt.float32
    f16 = mybir.dt.float16
    Erf = mybir.ActivationFunctionType.Erf
    mult = mybir.AluOpType.mult

    nc = bacc.Bacc(None, target_bir_lowering=False, name="gauss3d2")
    inp_d = nc.dram_tensor("inp", [128, _W_IN], f32, kind="ExternalInput")
    gyrep_d = nc.dram_tensor("gyrep", [128, NROWS * N_PIX], f16, kind="ExternalInput")
    gy48_d = nc.dram_tensor("gy48", [128, NROWS], f16, kind="ExternalInput")
    grid_d = nc.dram_tensor("grid", [128, SLAB * N_PIX], f16, kind="ExternalOutput")

    with tile.TileContext(nc) as tc:
        with (
            tc.tile_pool(name="const", bufs=1) as const,
            tc.tile_pool(name="work", bufs=3) as work,
            tc.tile_pool(name="o", bufs=2) as opool,
            tc.tile_pool(name="ps", bufs=1, space="PSUM") as psum,
        ):
            # trigger the Erf ACT table load first (input DMA flight overlap)
            warm = const.tile([128, 1], f32)
            nc.scalar.activation(
                warm[:], nc.const_aps.scalar_like(0.0, warm[:]), Erf
            )

            # input DMAs: f32 ctl+edges on sync, gy48 on scalar
            inp = const.tile([128, _W_IN], f32)
            nc.sync.dma_start(inp[:], inp_d[:])
            sedges = inp[:, _C_EDGE : _C_EDGE + 130]
            bxs = inp[:, _C_BX : _C_BX + NBLK]
            bzs = inp[:, _C_BZ : _C_BZ + NBLK]
            gy48 = const.tile([128, NROWS], f16, name="gy48")
            nc.scalar.dma_start(gy48[:], gy48_d[:])

            # gyrep[a, r, x] (contiguous fp16 -> DVE 2x H build). Block 1's
            # slice comes pre-broadcast from the host, split over both DMA
            # queues; blocks 2/3 are broadcast-built on ACT in its idle
            # window after the erfs (deterministic, no HBM-variance).
            gyrep = const.tile([128, NROWS, N_PIX], f16, name="gyrep")
            o1, r1b = _OFFS[1], _ROWS[1]
            h1 = r1b // 2
            nc.sync.dma_start(
                gyrep[:, o1 : o1 + h1, :],
                gyrep_d[:, o1 * N_PIX : (o1 + h1) * N_PIX],
            )
            nc.scalar.dma_start(
                gyrep[:, o1 + h1 : o1 + r1b, :],
                gyrep_d[:, (o1 + h1) * N_PIX : (o1 + r1b) * N_PIX],
            )

            # PE HAM warm-up fodder (no input deps): junk weights + rhs
            wgt = const.tile([128, 128], f16, name="wgt")
            wrhs = const.tile([128, 512], f16, name="wrhs")
            nc.gpsimd.memset(wgt[:], 0.0)
            nc.gpsimd.memset(wrhs[:], 0.0)
            pswarm = psum.tile([128, 512], f32, tag="pswarm", name="pswarm")
            for _ in range(12):
                nc.tensor.matmul(
                    pswarm[:], lhsT=wgt[:], rhs=wrhs[:],
                    start=True, stop=True, skip_group_check=True,
                )

            # 4 psum banks; banks 0,1 = y-rows 0-7 (written by B0,B1,B2),
            # banks 2,3 = y-rows 8-15 (written by B1,B2,B3)
            pss = [
                psum.tile([128, 512], f32, tag=f"ps{k}", name=f"ps{k}")
                for k in range(4)
            ]
            _PS_WRITERS = [
                [
                    b
                    for b in range(NBLK)
                    if RANGES[b][0] <= 4 * k and 4 * k + 4 <= RANGES[b][1]
                ]
                for k in range(4)
            ]
            shipped = set()

            def ship_bank(k):
                # copy engine alternates by bank; DMAs on both queues; the
                # final pair ships in halves so the last (receipt-latency-
                # bound) DMA is small and issued as early as possible
                eng = "a" if k in (0, 1, 2) else "v"
                ot = opool.tile([128, 512], f16, tag=f"ot{k}", name=f"ot{k}")
                q = nc.sync if eng == "a" else nc.scalar
                parts = ((0, 256), (256, 512)) if k >= 2 else ((0, 512),)
                for lo, hi in parts:
                    if eng == "a":
                        nc.scalar.copy(ot[:, lo:hi], pss[k][:, lo:hi])
                    else:
                        nc.vector.tensor_copy(ot[:, lo:hi], pss[k][:, lo:hi])
                    q.dma_start(
                        grid_d[:, 512 * k + lo : 512 * k + hi], ot[:, lo:hi]
                    )
                shipped.add(k)

            # all erfs first (ACT), so ACT's idle window after them can
            # build gyrep for blocks 2/3
            exzs = []
            for b in range(NBLK):
                exz = work.tile([128, 260], f32, tag=f"exz{b}", name=f"exz{b}")
                nc.scalar.activation(
                    exz[:, 0:130], sedges, Erf, bias=bxs[:, b : b + 1], scale=1.0
                )
                nc.scalar.activation(
                    exz[:, 130:260], sedges, Erf, bias=bzs[:, b : b + 1], scale=1.0
                )
                exzs.append(exz)
            for b in (2, 3):
                o, r = _OFFS[b], _ROWS[b]
                nc.scalar.copy(
                    gyrep[:, o : o + r, :],
                    gy48[:, o : o + r].broadcast_to([128, r, N_PIX]),
                )

            for b in range(NBLK):
                r0, r1 = RANGES[b]
                rows = r1 - r0
                exz = exzs[b]
                # diff: gx at [0:128], gz at [130:258] (junk at 128,129,258,259)
                gxz = work.tile([128, 260], f16, tag=f"gxz{b}", name=f"gxz{b}")
                nc.vector.tensor_sub(gxz[:, 0:259], exz[:, 1:260], exz[:, 0:259])
                gx = gxz[:, 0:N_PIX]
                gz = gxz[:, 130 : 130 + N_PIX]

                # H[a, r, x] = gx[a,x] * gy[a,r]
                h = work.tile([128, rows, N_PIX], f16, tag=f"h{b}", name=f"h{b}")
                o = _OFFS[b]
                if b == 0:
                    # 1x broadcast from the small gy48 (no bulk-DMA wait)
                    nc.vector.tensor_tensor(
                        h[:],
                        _bcast_mid(gx, rows),
                        gy48[:, o : o + rows].broadcast_to([128, rows, N_PIX]),
                        mult,
                    )
                else:
                    # DVE 2x packed mode via contiguous gyrep
                    nc.vector.tensor_tensor(
                        h[:], _bcast_mid(gx, rows), gyrep[:, o : o + rows, :], mult
                    )

                # matmuls: bank k covers y-rows [4k, 4k+4)
                for k in range(4):
                    if b not in _PS_WRITERS[k]:
                        continue
                    first = _PS_WRITERS[k][0] == b
                    last = _PS_WRITERS[k][-1] == b
                    lo = 4 * k - r0
                    nc.tensor.matmul(
                        pss[k][:],
                        lhsT=gz,
                        rhs=h[:, lo : lo + 4, :],
                        start=first,
                        stop=last,
                        skip_group_check=True,
                    )
                for k in range(4):
                    if k not in shipped and _PS_WRITERS[k][-1] == b:
                        ship_bank(k)

    nc.compile()
    return nc


def _shard_inputs(pos: np.ndarray, sigma: float, vs: float, n_pix: int, c_amp: float):
    """Per-core inputs: f32 [128, _W_IN] (bias + scaled edges) + f16 gy48."""
    from scipy.special import erf as serf

    inv_d = np.float64(1.0 / (np.sqrt(2.0) * sigma))
    edges = ((np.arange(n_pix + 1, dtype=np.float64) - n_pix // 2) - 0.5) * vs
    sedges = np.zeros((130,), np.float64)
    sedges[: n_pix + 1] = edges * inv_d
    sedges[n_pix + 1] = sedges[n_pix]  # pad col

    in_maps = []
    for i in range(N_CORES):
        lo, hi = edges[SLAB * i], edges[SLAB * i + SLAB]
        py = pos[:, 1].astype(np.float64)
        d = np.maximum(0.0, np.maximum(lo - py, py - hi))
        idx = np.argsort(d, kind="stable")[:CAP]
        idx = idx[d[idx] <= MAXDIST * sigma]

        centers = (np.arange(SLAB, dtype=np.float64) + SLAB * i - n_pix // 2) * vs
        p = py[idx]
        first = np.searchsorted(centers, p - SUPPORT * sigma, side="left")
        last = np.searchsorted(centers, p + SUPPORT * sigma, side="right")
        elig = [
            [
                b
                for b, (r0, r1) in enumerate(RANGES)
                if first[a] >= r0 and last[a] <= r1
            ]
            for a in range(len(idx))
        ]
        order = sorted(range(len(idx)), key=lambda a: len(elig[a]))
        cap = [128] * NBLK
        assign = [[] for _ in range(NBLK)]
        for a in order:
            cands = [b for b in elig[a] if cap[b] > 0]
            assert cands, f"core {i}: atom unassignable"
            b = min(cands, key=lambda b: RANGES[b][1] - RANGES[b][0])
            cap[b] -= 1
            assign[b].append(idx[a])

        buf = np.zeros((128, _W_IN), np.float32)
        buf[:, _C_EDGE : _C_EDGE + 130] = sedges[None, :].astype(np.float32)
        gy48 = np.zeros((128, NROWS), np.float16)
        yed = edges[SLAB * i : SLAB * i + SLAB + 1]
        for b in range(NBLK):
            r0, r1 = RANGES[b]
            sel = np.array(assign[b], dtype=np.int64)
            n = len(sel)
            px = np.zeros((128,), np.float64)
            pz = np.zeros((128,), np.float64)
            if n:
                px[:n] = pos[sel, 0]
                pz[:n] = pos[sel, 2]
                E = serf((yed[None, r0 : r1 + 1] - py[sel][:, None]) * inv_d)
                g = (E[:, 1:] - E[:, :-1]) * c_amp  # c_amp = amp*(0.5/vs)^3
                gy48[:n, _OFFS[b] : _OFFS[b] + (r1 - r0)] = g.astype(np.float16)
            buf[:, _C_BX + b] = (-px * inv_d).astype(np.float32)
            buf[:, _C_BZ + b] = (-pz * inv_d).astype(np.float32)
        gyrep = np.repeat(gy48[:, :, None], N_PIX, axis=2).reshape(128, NROWS * N_PIX)
        in_maps.append({"inp": buf, "gyrep": gyrep, "gy48": gy48})
    return in_maps


def kernel(
    atom_positions: np.ndarray,
    log_var: np.ndarray,
    log_weight: np.ndarray,
    n_pix,
    voxel_size,
) -> np.ndarray:
    global LAST_RESULTS
    pos = np.asarray(atom_positions, dtype=np.float32)
    lv = float(np.asarray(log_var, dtype=np.float32).reshape(-1)[0])
    lw = float(np.asarray(log_weight, dtype=np.float32).reshape(-1)[0])
    n_pix = int(n_pix)
    vs = float(voxel_size)
    assert n_pix == N_PIX, f"kernel compiled for n_pix={N_PIX}, got {n_pix}"

    sigma = float(np.exp(0.5 * lv))
    amp = float(np.exp(lw))
    c_amp = float(amp * (0.5 / vs) ** 3)  # folded into gy on host

    in_maps = _shard_inputs(pos, sigma, vs, n_pix, c_amp)
    nc = _build_nc()
    res = run_bass_kernel_spmd(
        nc,
        in_maps,
        core_ids=list(range(N_CORES)),
        trace=bool(int(os.environ.get("GAUSS3D_TRACE", "0"))),
    )
    LAST_RESULTS = res
    grids = [
        r["grid"].astype(np.float32).reshape(N_PIX, SLAB, N_PIX) for r in res.results
    ]
    return np.ascontiguousarray(np.concatenate(grids, axis=1), dtype=np.float32)
